# revision 33
# baseline (speedup 1.0000x reference)
"""Bahdanau-attention kernel for 8 TRN2 NeuronCores.

Reference computation (B=32, S=2048, H=1024):
    eo   = encoder_outputs.transpose(1,0,2)            # [B,S,H]
    z    = hidden @ W[:, :H].T + eo @ W[:, H:].T + b   # [B,S,H]  (split concat)
    s    = tanh(z)
    sc   = einsum('bsh,h->bs', s, v)
    sc   = where(mask, -1e9, sc); softmax over S       # [B,1,S]

Device work is the irreducible nonlinear core: z8 = w8 @ e8 (fp8 e4m3
DoubleRow matmuls, 2 k-tiles per instruction at double rate), tanh with
the hidden-path bias fused (ScalarE), the v-weighted accumulate
(VectorE, bf16 2x) reduced across partitions by a ones-matmul, then a
masked exp.  Normalization happens on the host (exp rows + partial sums
are the outputs).

Everything linear in the inputs is precomputed exactly on the host and
injected as bias rows:
  * pre[b,h]  = hidden @ Wh^T + b          (tanh per-partition bias)
  * c[b,s]    = u.eo - u8.e8  with u = We^T v, u8 = dequant(w8)^T vb
    -- the exact linear error of the fp8 z-path, added to the score row
    (folded into the same row that kills padding columns with -1e30).
score = v.tanh(z8) + c reproduces the reference to ~1e-2 of max output.

Mask-skip: masked positions softmax to exactly 0 in fp32, so only
unmasked columns are packed (host gather), computed, and scattered back.

Sharding: data-parallel over batch, 4 batches per core.  Batches are
assigned to (core, slot) by sorted unmasked-count so that the padded
per-slot capacity (shared across cores by the SPMD program) is tight:
slot k's capacity is the max count among its 8 batches.
"""

import sys

if "/opt/trn_rl_repo" not in sys.path:
    sys.path.insert(0, "/opt/trn_rl_repo")

import numpy as np

B, S, H = 32, 2048, 1024
NCORES = 8
BL = B // NCORES          # batches per core = 4
P = 128                   # partitions
KT = H // P               # k-tiles over the contraction dim = 8
KP = KT // 2              # DoubleRow k-tile pairs = 4
HT = H // P               # h-tiles over the attn output dim = 8
SE = 16.0                 # eo fp8 scale
SW = 32.0                 # We fp8 scale
ZS = 1.0 / (SE * SW)      # psum -> z units

MAXC = 512                # max chunk width (psum bank, fp32)
NWARM = 14                # PE warmup matmuls (p-state ramp + head DMA)

_compiled = {}


def _balanced(cap):
    nch = -(-cap // MAXC)
    base = -(-cap // (nch * 8)) * 8
    widths = [base] * (nch - 1)
    widths.append(cap - base * (nch - 1))
    assert all(0 < w <= MAXC for w in widths) and sum(widths) == cap
    return widths


def _layout(segs):
    """Static schedule shared by _build and run: processing order, chunk
    list, per-slot stream offsets, total stream length."""
    # process last the slot whose final max-first chunk is smallest, so
    # the tail chain is short
    proc = sorted(range(BL), key=lambda k: -((segs[k] - 1) % MAXC + 1))
    chunks = []               # (slot, stream_c0, slot_c0, width)
    offs = {}
    pos = 0
    for k in proc:
        offs[k] = pos
        widths = _balanced(segs[k])
        c0 = 0
        for w in widths:
            chunks.append((k, pos + c0, c0, w))
            c0 += w
        pos += segs[k]
    return proc, chunks, offs, pos


def _build(segs):
    import concourse.mybir as mybir
    from concourse import tile, bacc
    from concourse.tile import add_dep_helper

    f32 = mybir.dt.float32
    bf16 = mybir.dt.bfloat16
    fp8 = mybir.dt.float8e4
    AF = mybir.ActivationFunctionType
    ALU = mybir.AluOpType
    DR = mybir.MatmulPerfMode.DoubleRow

    proc, chunks, soffs, tot = _layout(segs)
    nchk = len(chunks)

    nc = bacc.Bacc("TRN2", target_bir_lowering=False, debug=False,
                   num_devices=NCORES)

    eo8d = [nc.dram_tensor(f"eo8_{k}", [P, KP, 2, segs[k]], fp8,
                           kind="ExternalInput") for k in range(BL)]
    w8st = nc.dram_tensor("w8st", [P, HT, KP, 2, P], fp8,
                          kind="ExternalInput")
    vsc = nc.dram_tensor("vsc", [P, HT], f32, kind="ExternalInput")
    prer = nc.dram_tensor("prer", [P, HT * BL], f32, kind="ExternalInput")
    padc = nc.dram_tensor("padc", [1, tot], f32, kind="ExternalInput")
    padc8 = nc.dram_tensor("padc8", [1, tot], bf16, kind="ExternalInput")
    eout = nc.dram_tensor("eout", [1, tot], f32, kind="ExternalOutput")

    with tile.TileContext(nc) as tc:
        with (
            tc.tile_pool(name="const", bufs=1) as const,
            tc.tile_pool(name="tpool", bufs=18) as t_pool,
            tc.tile_pool(name="accpool", bufs=5) as acc_pool,
            tc.tile_pool(name="scpool", bufs=3) as sc_pool,
            tc.tile_pool(name="psz", bufs=5, space="PSUM") as psum_z,
            tc.tile_pool(name="pss", bufs=3, space="PSUM") as psum_s,
        ):
            # --- weights first, split per-hh so z(0) can start after
            # 128KB; the eo slots stream on the SWDGE ring concurrently.
            # Tiny consts ride behind the first weight slices. ---
            w8_sb = const.tile([P, HT, KP, 2, P], fp8)
            eo_sbs = [const.tile([P, KP, 2, segs[k]], fp8, name=f"eo_sb{k}")
                      for k in range(BL)]
            for hh in range(HT):
                nc.sync.dma_start(w8_sb[:, hh], w8st[:, hh])
            # serialize the eo slot streams so the concurrent w8 slices
            # keep their share of HBM bandwidth at the head
            prev = None
            for ki, k in enumerate(proc):
                if ki == 0:
                    # first slot in two column-halves so z(0) starts early
                    h1 = segs[k] // 2
                    prev = nc.gpsimd.dma_start(eo_sbs[k][:, :, :, :h1],
                                               eo8d[k][:, :, :, :h1])
                    d = nc.gpsimd.dma_start(eo_sbs[k][:, :, :, h1:],
                                            eo8d[k][:, :, :, h1:])
                else:
                    d = nc.gpsimd.dma_start(eo_sbs[k][:], eo8d[k][:])
                add_dep_helper(d.ins, prev.ins, True, "serial eo dma")
                prev = d

            vsc_sb = const.tile([P, HT], f32)
            nc.sync.dma_start(vsc_sb[:], vsc[:, :])
            vsc8_sb = const.tile([P, HT], bf16)
            nc.vector.tensor_copy(vsc8_sb[:], vsc_sb[:])
            pre_sb = const.tile([P, HT * BL], f32)
            nc.sync.dma_start(pre_sb[:], prer[:, :])
            padc_sb = const.tile([1, tot], f32)
            nc.sync.dma_start(padc_sb[:], padc[:, :])

            ones_sb = const.tile([P, 1], bf16)
            nc.any.memset(ones_sb[:], 1.0)
            junk = const.tile([P, MAXC], bf16)
            nc.vector.tensor_copy(junk[:, 0:1], ones_sb[:])

            # PE warmup: ride out the p-state ramp while the head DMAs land
            wps = psum_z.tile([P, MAXC], f32, tag="psz")
            for w in range(NWARM):
                nc.tensor.matmul(wps[:], junk[:, 0:P], junk[:],
                                 start=(w == 0), stop=(w == NWARM - 1),
                                 skip_group_check=True)

            e_row = const.tile([1, tot], f32)
            padc8_sb = const.tile([1, tot], bf16)
            nc.sync.dma_start(padc8_sb[:], padc8[:, :])

            # chunks awaiting their ones-matmul reduce; flushed inside the
            # NEXT chunk's z-groups so the PE never stalls on the
            # tanh/vector chain.
            pending = []

            def emit_exp(gci, sc0, wc, acc_psum):
                sc_m = sc_pool.tile([1, wc], f32, tag="sc", name="sc_m")
                nc.vector.tensor_tensor(sc_m[:], acc_psum,
                                        padc_sb[:, sc0:sc0 + wc], ALU.add)
                nc.scalar.activation(e_row[:, sc0:sc0 + wc], sc_m[:],
                                     AF.Exp)

            def flush_pending():
                for acc, gci, sc0, wc in pending:
                    pss = psum_s.tile([1, MAXC], f32, tag="pss", name="pss")
                    nc.tensor.matmul(pss[:1, :wc], ones_sb[:], acc[:],
                                     start=True, stop=True,
                                     skip_group_check=True)
                    emit_exp(gci, sc0, wc, pss[:1, :wc])
                pending.clear()

            for gci, (k, sc0, kc0, wc) in enumerate(chunks):
                eo_sb = eo_sbs[k]
                cs = slice(kc0, kc0 + wc)
                tail = gci == nchk - 1
                if tail:
                    pss_t = psum_s.tile([1, MAXC], f32, tag="pss",
                                        name="pss_t")
                    t8s = []
                else:
                    acc = acc_pool.tile([P, wc], bf16, tag="acc", name="acc")
                for hh in range(HT):
                    zp = psum_z.tile([P, wc], f32, tag="psz", name="zp")
                    for j in range(KP):
                        nc.tensor.matmul(
                            zp[:], w8_sb[:, hh, j, :, :],
                            eo_sb[:, j, :, cs], start=(j == 0),
                            stop=(j == KP - 1), perf_mode=DR)
                    if hh == 2 and pending:
                        flush_pending()
                        if gci == nchk - 1:
                            # every non-tail chunk's exp row is final now
                            nc.sync.dma_start(eout[:, 0:sc0],
                                              e_row[:, 0:sc0])
                    t8 = t_pool.tile([P, wc], bf16, tag="t", name="t8")
                    nc.scalar.activation(
                        t8[:], zp[:], AF.Tanh, scale=ZS,
                        bias=pre_sb[:, hh * BL + k:hh * BL + k + 1])
                    if tail:
                        t8s.append(t8)
                        if hh >= 2:
                            nc.tensor.matmul(
                                pss_t[:1, :wc], vsc8_sb[:, hh - 2:hh - 1],
                                t8s[hh - 2][:], start=(hh == 2), stop=False,
                                skip_group_check=True)
                    elif hh == 0:
                        nc.vector.tensor_scalar(acc[:], t8[:],
                                                vsc_sb[:, 0:1], None,
                                                ALU.mult)
                    else:
                        tv = t_pool.tile([P, wc], bf16, tag="tv", name="tv")
                        nc.vector.tensor_scalar(tv[:], t8[:],
                                                vsc_sb[:, hh:hh + 1],
                                                None, ALU.mult)
                        nc.vector.tensor_tensor(acc[:], acc[:], tv[:],
                                                ALU.add)
                if tail:
                    for h2 in range(HT - 2, HT):
                        nc.tensor.matmul(
                            pss_t[:1, :wc], vsc8_sb[:, h2:h2 + 1],
                            t8s[h2][:], start=False, stop=(h2 == HT - 1),
                            skip_group_check=True)
                    nc.tensor.matmul(pss_t[:1, :wc], ones_sb[0:1, 0:1],
                                     padc8_sb[:, sc0:sc0 + wc], start=False,
                                     stop=True, skip_group_check=True)
                    nc.scalar.activation(e_row[:, sc0:sc0 + wc],
                                         pss_t[:1, :wc], AF.Exp)
                    nc.sync.dma_start(eout[:, sc0:], e_row[:, sc0:])
                else:
                    pending.append((acc, gci, sc0, wc))
            flush_pending()

    nc.compile()
    return nc


def _get_nc(segs=(1072, 1048, 1032, 1024)):
    segs = tuple(segs)
    if segs not in _compiled:
        _compiled[segs] = _build(segs)
    return _compiled[segs]


def _prep(hidden, encoder_outputs, encoder_mask, W, b, v):
    """Host-side packing/quantization. Returns (in_maps, scatter_info)."""
    import ml_dtypes

    bf16 = ml_dtypes.bfloat16
    f8 = ml_dtypes.float8_e4m3

    hidden = np.asarray(hidden, dtype=np.float32)
    eo = np.asarray(encoder_outputs, dtype=np.float32)      # [S, B, H]
    W = np.asarray(W, dtype=np.float32)
    bias = np.asarray(b, dtype=np.float32)
    v = np.asarray(v, dtype=np.float32)
    mask = np.asarray(encoder_mask).reshape(B, S)

    Wh, We = W[:, :H], W[:, H:]

    w8 = (We * SW).astype(f8)
    w8f = w8.astype(np.float32)
    vb = v.astype(bf16).astype(np.float32)
    u = (We.T @ v).astype(np.float32)            # exact linear weights
    u8 = (w8f / SW).T @ vb                       # device linear weights

    pre = hidden @ Wh.T + bias                   # [B, H] exact hidden path

    # batch -> (core, slot) assignment by sorted unmasked count: slot k's
    # capacity = max count among its 8 batches, uniform across cores
    idxs = [np.nonzero(mask[gb] == 0)[0] for gb in range(B)]
    ns = np.array([len(ix) for ix in idxs])
    order = np.argsort(-ns, kind="stable")
    assign = order.reshape(BL, NCORES)           # assign[k][c] = global batch
    segs = tuple(max(8, -(-int(ns[assign[k]].max()) // 8) * 8)
                 for k in range(BL))

    w8st = np.ascontiguousarray(
        w8.T.reshape(KP, 2, P, HT, P).transpose(2, 3, 0, 1, 4))
    vsc = np.ascontiguousarray(
        v.astype(bf16).astype(np.float32).reshape(HT, P).T)

    proc, chunks, soffs, tot = _layout(segs)

    in_maps = []
    for c in range(NCORES):
        padc = np.full((tot,), -1e30, dtype=np.float32)
        pre_r = np.empty((BL, HT, P), dtype=np.float32)
        im = {"w8st": w8st, "vsc": vsc}
        for k in range(BL):
            gb = int(assign[k][c])
            ix = idxs[gb]
            n = len(ix)
            eo8c = np.zeros((P, KP, 2, segs[k]), dtype=f8)
            ecols = np.ascontiguousarray(eo[ix, gb, :].T)   # [H, n]
            e8 = (ecols * SE).astype(f8)
            eo8c[:, :, :, :n] = e8.reshape(KP, 2, P, n).transpose(2, 0, 1, 3)
            im[f"eo8_{k}"] = eo8c
            padc[soffs[k]:soffs[k] + n] = \
                u @ ecols - (u8 @ e8.astype(np.float32)) / SE
            pre_r[k] = pre[gb].reshape(HT, P)
        im["prer"] = np.ascontiguousarray(
            pre_r.transpose(2, 1, 0).reshape(P, HT * BL))
        im["padc"] = padc.reshape(1, tot)
        im["padc8"] = padc.reshape(1, tot).astype(bf16)
        in_maps.append(im)
    return in_maps, (idxs, ns, assign, segs, chunks, soffs, tot)


def run(hidden, encoder_outputs, encoder_mask, W, b, v, trace=False):
    from concourse.bass_utils import run_bass_kernel_spmd

    in_maps, meta = _prep(hidden, encoder_outputs, encoder_mask, W, b, v)
    idxs, ns, assign, segs, chunks, soffs, tot = meta
    nc = _get_nc(segs)
    res = run_bass_kernel_spmd(nc, in_maps, core_ids=list(range(NCORES)),
                               trace=trace)
    full = np.zeros((B, S), dtype=np.float32)
    for c in range(NCORES):
        e = res.results[c]["eout"].ravel()
        for k in range(BL):
            gb = int(assign[k][c])
            if ns[gb] == 0:
                full[gb, :] = 1.0 / S     # all masked: softmax is uniform
                continue
            ek = e[soffs[k]:soffs[k] + ns[gb]]
            full[gb, idxs[gb]] = ek / ek.sum(dtype=np.float64)
    return full.reshape(B, 1, S), res


def kernel(hidden, encoder_outputs, encoder_mask, W, b, v):
    out, _ = run(hidden, encoder_outputs, encoder_mask, W, b, v, trace=False)
    return out


# revision 34
# speedup vs baseline: 1.0092x; 1.0092x over previous
"""Bahdanau-attention kernel for 8 TRN2 NeuronCores.

Reference computation (B=32, S=2048, H=1024):
    eo   = encoder_outputs.transpose(1,0,2)            # [B,S,H]
    z    = hidden @ W[:, :H].T + eo @ W[:, H:].T + b   # [B,S,H]  (split concat)
    s    = tanh(z)
    sc   = einsum('bsh,h->bs', s, v)
    sc   = where(mask, -1e9, sc); softmax over S       # [B,1,S]

Device work is the irreducible nonlinear core: z8 = w8 @ e8 (fp8 e4m3
DoubleRow matmuls, 2 k-tiles per instruction at double rate), tanh with
the hidden-path bias fused (ScalarE), the v-weighted accumulate
(VectorE, bf16 2x) reduced across partitions by a ones-matmul, then a
masked exp.  Normalization happens on the host (exp rows + partial sums
are the outputs).

Everything linear in the inputs is precomputed exactly on the host and
injected as bias rows:
  * pre[b,h]  = hidden @ Wh^T + b          (tanh per-partition bias)
  * c[b,s]    = u.eo - u8.e8  with u = We^T v, u8 = dequant(w8)^T vb
    -- the exact linear error of the fp8 z-path, added to the score row
    (folded into the same row that kills padding columns with -1e30).
score = v.tanh(z8) + c reproduces the reference to ~1e-2 of max output.

Mask-skip: masked positions softmax to exactly 0 in fp32, so only
unmasked columns are packed (host gather), computed, and scattered back.

Sharding: data-parallel over batch, 4 batches per core.  Batches are
assigned to (core, slot) by sorted unmasked-count so that the padded
per-slot capacity (shared across cores by the SPMD program) is tight:
slot k's capacity is the max count among its 8 batches.
"""

import sys

if "/opt/trn_rl_repo" not in sys.path:
    sys.path.insert(0, "/opt/trn_rl_repo")

import numpy as np

B, S, H = 32, 2048, 1024
NCORES = 8
BL = B // NCORES          # batches per core = 4
P = 128                   # partitions
KT = H // P               # k-tiles over the contraction dim = 8
KP = KT // 2              # DoubleRow k-tile pairs = 4
HT = H // P               # h-tiles over the attn output dim = 8
SE = 16.0                 # eo fp8 scale
SW = 32.0                 # We fp8 scale
ZS = 1.0 / (SE * SW)      # psum -> z units

MAXC = 512                # max chunk width (psum bank, fp32)
NWARM = 12                # PE warmup matmuls (p-state ramp + head DMA)

_compiled = {}


def _balanced(cap):
    nch = -(-cap // MAXC)
    base = -(-cap // (nch * 8)) * 8
    widths = [base] * (nch - 1)
    widths.append(cap - base * (nch - 1))
    assert all(0 < w <= MAXC for w in widths) and sum(widths) == cap
    return widths


def _layout(segs):
    """Static schedule shared by _build and run: processing order, chunk
    list, per-slot stream offsets, total stream length."""
    # process last the slot whose final max-first chunk is smallest, so
    # the tail chain is short
    proc = sorted(range(BL), key=lambda k: -((segs[k] - 1) % MAXC + 1))
    chunks = []               # (slot, stream_c0, slot_c0, width)
    offs = {}
    pos = 0
    for k in proc:
        offs[k] = pos
        widths = _balanced(segs[k])
        c0 = 0
        for w in widths:
            chunks.append((k, pos + c0, c0, w))
            c0 += w
        pos += segs[k]
    return proc, chunks, offs, pos


def _build(segs):
    import concourse.mybir as mybir
    from concourse import tile, bacc
    from concourse.tile import add_dep_helper

    f32 = mybir.dt.float32
    bf16 = mybir.dt.bfloat16
    fp8 = mybir.dt.float8e4
    AF = mybir.ActivationFunctionType
    ALU = mybir.AluOpType
    DR = mybir.MatmulPerfMode.DoubleRow

    proc, chunks, soffs, tot = _layout(segs)
    nchk = len(chunks)

    nc = bacc.Bacc("TRN2", target_bir_lowering=False, debug=False,
                   num_devices=NCORES)

    eo8d = [nc.dram_tensor(f"eo8_{k}", [P, KP, 2, segs[k]], fp8,
                           kind="ExternalInput") for k in range(BL)]
    w8st = nc.dram_tensor("w8st", [P, HT, KP, 2, P], fp8,
                          kind="ExternalInput")
    vsc = nc.dram_tensor("vsc", [P, HT], f32, kind="ExternalInput")
    prer = nc.dram_tensor("prer", [P, HT * BL], f32, kind="ExternalInput")
    padc = nc.dram_tensor("padc", [1, tot], f32, kind="ExternalInput")
    padc8 = nc.dram_tensor("padc8", [1, tot], bf16, kind="ExternalInput")
    eout = nc.dram_tensor("eout", [1, tot], f32, kind="ExternalOutput")

    with tile.TileContext(nc) as tc:
        with (
            tc.tile_pool(name="const", bufs=1) as const,
            tc.tile_pool(name="tpool", bufs=18) as t_pool,
            tc.tile_pool(name="accpool", bufs=5) as acc_pool,
            tc.tile_pool(name="scpool", bufs=3) as sc_pool,
            tc.tile_pool(name="psz", bufs=5, space="PSUM") as psum_z,
            tc.tile_pool(name="pss", bufs=3, space="PSUM") as psum_s,
        ):
            # --- weights first, split per-hh so z(0) can start after
            # 128KB; the eo slots stream on the SWDGE ring concurrently.
            # Tiny consts ride behind the first weight slices. ---
            w8_sb = const.tile([P, HT, KP, 2, P], fp8)
            eo_sbs = [const.tile([P, KP, 2, segs[k]], fp8, name=f"eo_sb{k}")
                      for k in range(BL)]
            for hh in range(HT):
                nc.sync.dma_start(w8_sb[:, hh], w8st[:, hh])
            # serialize the eo slot streams so the concurrent w8 slices
            # keep their share of HBM bandwidth at the head
            prev = None
            for ki, k in enumerate(proc):
                if ki == 0:
                    # first slot in two column-halves so z(0) starts early
                    h1 = segs[k] // 2
                    prev = nc.gpsimd.dma_start(eo_sbs[k][:, :, :, :h1],
                                               eo8d[k][:, :, :, :h1])
                    d = nc.gpsimd.dma_start(eo_sbs[k][:, :, :, h1:],
                                            eo8d[k][:, :, :, h1:])
                else:
                    d = nc.gpsimd.dma_start(eo_sbs[k][:], eo8d[k][:])
                add_dep_helper(d.ins, prev.ins, True, "serial eo dma")
                prev = d

            vsc_sb = const.tile([P, HT], f32)
            nc.sync.dma_start(vsc_sb[:], vsc[:, :])
            vsc8_sb = const.tile([P, HT], bf16)
            nc.vector.tensor_copy(vsc8_sb[:], vsc_sb[:])
            pre_sb = const.tile([P, HT * BL], f32)
            nc.sync.dma_start(pre_sb[:], prer[:, :])
            padc_sb = const.tile([1, tot], f32)
            nc.sync.dma_start(padc_sb[:], padc[:, :])

            ones_sb = const.tile([P, 1], bf16)
            nc.any.memset(ones_sb[:], 1.0)
            junk = const.tile([P, MAXC], bf16)
            nc.vector.tensor_copy(junk[:, 0:1], ones_sb[:])
            # preload the tanh/exp activation table off the critical path
            actwarm = const.tile([1, 1], f32)
            nc.scalar.activation(actwarm[:], ones_sb[0:1, 0:1], AF.Tanh)

            # PE warmup: ride out the p-state ramp while the head DMAs land
            wps = psum_z.tile([P, MAXC], f32, tag="psz")
            for w in range(NWARM):
                nc.tensor.matmul(wps[:], junk[:, 0:P], junk[:],
                                 start=(w == 0), stop=(w == NWARM - 1),
                                 skip_group_check=True)

            e_row = const.tile([1, tot], f32)
            padc8_sb = const.tile([1, tot], bf16)
            nc.sync.dma_start(padc8_sb[:], padc8[:, :])

            # chunks awaiting their ones-matmul reduce; flushed inside the
            # NEXT chunk's z-groups so the PE never stalls on the
            # tanh/vector chain.
            pending = []

            def emit_exp(gci, sc0, wc, acc_psum):
                sc_m = sc_pool.tile([1, wc], f32, tag="sc", name="sc_m")
                nc.vector.tensor_tensor(sc_m[:], acc_psum,
                                        padc_sb[:, sc0:sc0 + wc], ALU.add)
                nc.scalar.activation(e_row[:, sc0:sc0 + wc], sc_m[:],
                                     AF.Exp)

            def flush_pending():
                for acc, gci, sc0, wc in pending:
                    pss = psum_s.tile([1, MAXC], f32, tag="pss", name="pss")
                    nc.tensor.matmul(pss[:1, :wc], ones_sb[:], acc[:],
                                     start=True, stop=True,
                                     skip_group_check=True)
                    emit_exp(gci, sc0, wc, pss[:1, :wc])
                pending.clear()

            for gci, (k, sc0, kc0, wc) in enumerate(chunks):
                eo_sb = eo_sbs[k]
                cs = slice(kc0, kc0 + wc)
                tail = gci == nchk - 1
                if tail:
                    pss_t = psum_s.tile([1, MAXC], f32, tag="pss",
                                        name="pss_t")
                    t8s = []
                else:
                    acc = acc_pool.tile([P, wc], bf16, tag="acc", name="acc")
                for hh in range(HT):
                    zp = psum_z.tile([P, wc], f32, tag="psz", name="zp")
                    for j in range(KP):
                        nc.tensor.matmul(
                            zp[:], w8_sb[:, hh, j, :, :],
                            eo_sb[:, j, :, cs], start=(j == 0),
                            stop=(j == KP - 1), perf_mode=DR)
                    if hh == 2 and pending:
                        flush_pending()
                        if gci == nchk - 1:
                            # every non-tail chunk's exp row is final now
                            nc.sync.dma_start(eout[:, 0:sc0],
                                              e_row[:, 0:sc0])
                    t8 = t_pool.tile([P, wc], bf16, tag="t", name="t8")
                    nc.scalar.activation(
                        t8[:], zp[:], AF.Tanh, scale=ZS,
                        bias=pre_sb[:, hh * BL + k:hh * BL + k + 1])
                    if tail:
                        t8s.append(t8)
                        if hh >= 2:
                            nc.tensor.matmul(
                                pss_t[:1, :wc], vsc8_sb[:, hh - 2:hh - 1],
                                t8s[hh - 2][:], start=(hh == 2), stop=False,
                                skip_group_check=True)
                    elif hh == 0:
                        nc.vector.tensor_scalar(acc[:], t8[:],
                                                vsc_sb[:, 0:1], None,
                                                ALU.mult)
                    else:
                        tv = t_pool.tile([P, wc], bf16, tag="tv", name="tv")
                        nc.vector.tensor_scalar(tv[:], t8[:],
                                                vsc_sb[:, hh:hh + 1],
                                                None, ALU.mult)
                        nc.vector.tensor_tensor(acc[:], acc[:], tv[:],
                                                ALU.add)
                if tail:
                    for h2 in range(HT - 2, HT):
                        nc.tensor.matmul(
                            pss_t[:1, :wc], vsc8_sb[:, h2:h2 + 1],
                            t8s[h2][:], start=False, stop=(h2 == HT - 1),
                            skip_group_check=True)
                    nc.tensor.matmul(pss_t[:1, :wc], ones_sb[0:1, 0:1],
                                     padc8_sb[:, sc0:sc0 + wc], start=False,
                                     stop=True, skip_group_check=True)
                    nc.scalar.activation(e_row[:, sc0:sc0 + wc],
                                         pss_t[:1, :wc], AF.Exp)
                    nc.sync.dma_start(eout[:, sc0:], e_row[:, sc0:])
                else:
                    pending.append((acc, gci, sc0, wc))
            flush_pending()

    nc.compile()
    return nc


def _get_nc(segs=(1072, 1048, 1032, 1024)):
    segs = tuple(segs)
    if segs not in _compiled:
        _compiled[segs] = _build(segs)
    return _compiled[segs]


def _prep(hidden, encoder_outputs, encoder_mask, W, b, v):
    """Host-side packing/quantization. Returns (in_maps, scatter_info)."""
    import ml_dtypes

    bf16 = ml_dtypes.bfloat16
    f8 = ml_dtypes.float8_e4m3

    hidden = np.asarray(hidden, dtype=np.float32)
    eo = np.asarray(encoder_outputs, dtype=np.float32)      # [S, B, H]
    W = np.asarray(W, dtype=np.float32)
    bias = np.asarray(b, dtype=np.float32)
    v = np.asarray(v, dtype=np.float32)
    mask = np.asarray(encoder_mask).reshape(B, S)

    Wh, We = W[:, :H], W[:, H:]

    w8 = (We * SW).astype(f8)
    w8f = w8.astype(np.float32)
    vb = v.astype(bf16).astype(np.float32)
    u = (We.T @ v).astype(np.float32)            # exact linear weights
    u8 = (w8f / SW).T @ vb                       # device linear weights

    pre = hidden @ Wh.T + bias                   # [B, H] exact hidden path

    # batch -> (core, slot) assignment by sorted unmasked count: slot k's
    # capacity = max count among its 8 batches, uniform across cores
    idxs = [np.nonzero(mask[gb] == 0)[0] for gb in range(B)]
    ns = np.array([len(ix) for ix in idxs])
    order = np.argsort(-ns, kind="stable")
    assign = order.reshape(BL, NCORES)           # assign[k][c] = global batch
    segs = tuple(max(8, -(-int(ns[assign[k]].max()) // 8) * 8)
                 for k in range(BL))

    w8st = np.ascontiguousarray(
        w8.T.reshape(KP, 2, P, HT, P).transpose(2, 3, 0, 1, 4))
    vsc = np.ascontiguousarray(
        v.astype(bf16).astype(np.float32).reshape(HT, P).T)

    proc, chunks, soffs, tot = _layout(segs)

    in_maps = []
    for c in range(NCORES):
        padc = np.full((tot,), -1e30, dtype=np.float32)
        pre_r = np.empty((BL, HT, P), dtype=np.float32)
        im = {"w8st": w8st, "vsc": vsc}
        for k in range(BL):
            gb = int(assign[k][c])
            ix = idxs[gb]
            n = len(ix)
            eo8c = np.zeros((P, KP, 2, segs[k]), dtype=f8)
            ecols = np.ascontiguousarray(eo[ix, gb, :].T)   # [H, n]
            e8 = (ecols * SE).astype(f8)
            eo8c[:, :, :, :n] = e8.reshape(KP, 2, P, n).transpose(2, 0, 1, 3)
            im[f"eo8_{k}"] = eo8c
            padc[soffs[k]:soffs[k] + n] = \
                u @ ecols - (u8 @ e8.astype(np.float32)) / SE
            pre_r[k] = pre[gb].reshape(HT, P)
        im["prer"] = np.ascontiguousarray(
            pre_r.transpose(2, 1, 0).reshape(P, HT * BL))
        im["padc"] = padc.reshape(1, tot)
        im["padc8"] = padc.reshape(1, tot).astype(bf16)
        in_maps.append(im)
    return in_maps, (idxs, ns, assign, segs, chunks, soffs, tot)


def run(hidden, encoder_outputs, encoder_mask, W, b, v, trace=False):
    from concourse.bass_utils import run_bass_kernel_spmd

    in_maps, meta = _prep(hidden, encoder_outputs, encoder_mask, W, b, v)
    idxs, ns, assign, segs, chunks, soffs, tot = meta
    nc = _get_nc(segs)
    res = run_bass_kernel_spmd(nc, in_maps, core_ids=list(range(NCORES)),
                               trace=trace)
    full = np.zeros((B, S), dtype=np.float32)
    for c in range(NCORES):
        e = res.results[c]["eout"].ravel()
        for k in range(BL):
            gb = int(assign[k][c])
            if ns[gb] == 0:
                full[gb, :] = 1.0 / S     # all masked: softmax is uniform
                continue
            ek = e[soffs[k]:soffs[k] + ns[gb]]
            full[gb, idxs[gb]] = ek / ek.sum(dtype=np.float64)
    return full.reshape(B, 1, S), res


def kernel(hidden, encoder_outputs, encoder_mask, W, b, v):
    out, _ = run(hidden, encoder_outputs, encoder_mask, W, b, v, trace=False)
    return out
